# revision 10
# baseline (speedup 1.0000x reference)
"""Trainium2 Bass kernel for a char-CNN (embed lookup + conv1d(K=5,pad=2) + bias + maxpool).

Math: out[n, f] = max_w ( b[f] + sum_k sum_d  E[ids[n, w+k-2], d] * Wc[f, d, k] )

Strategy (pure data-parallel over 8 cores, 4096 tokens each):
  * Host-side constant folding (weights only): G[k][v, f] = sum_d E[v, d] * Wc[f, d, k].
    The embedding+conv collapses to y[n,:,w] = sum_k G[k][ids[n,w+k-2], :] + b.
  * On device, table lookup runs on the TensorEngine as one-hot matmuls with
    contraction over the vocab (96) plus a constant-ones row that carries the bias:
      - ids arrive via DMA transposed to (w, token) order per 32-token half, so
        every downstream access pattern is sequential
      - broadcast ids across partitions with K=1 ones-matmuls   -> psum [96, cols]
      - one-hot = is_equal(bcast, iota_per_partition) on VectorE, written into a
        padded [vocab+1, half, W+5 slots, 32 tokens] layout (f32r, slot-major) so
        the 5 shifted tap reads are flat contiguous and token edges see zeros
      - hi pass: 5 taps of f32r matmuls against G_hi (f32r keeps 12 mantissa
        bits), PSUM-accumulated
      - lo pass: the f32r residual, scaled by 4096 and quantized to fp8-e4m3,
        runs as 3 DoubleRow fp8 matmuls (two taps = two contraction k-tiles per
        pass).  The moving operand is an e5m2 one-hot carrying the 2^-12
        unscale, produced from the f32r one-hot by the (otherwise idle) Scalar
        engine as two tap-shifted planes.
      - reduce_max over the 16 positions on VectorE
  * The broadcast/one-hot/fp8-convert for unit u+1 is emitted before unit u's
    taps so the in-order PE queue never stalls on the VectorE.
  * Output is produced as [group, F, 512] per core; host transposes/concats.
"""

import numpy as np

import concourse.bass as bass
import concourse.bacc as bacc
import concourse.mybir as mybir
from concourse.tile import TileContext
from concourse.bass_utils import run_bass_kernel_spmd

# Problem shapes (hardcoded per contract)
N, W = 32768, 16
VOCAB, D, F, K = 96, 100, 100, 5
N_CORES = 8
NSH = N // N_CORES            # tokens per core = 4096
UNIT = 64                     # tokens per pipeline unit (=> 1024 one-hot cols)
NUNIT = NSH // UNIT           # 64
GROUP = 512                   # tokens per ids DMA
NGROUP = NSH // GROUP         # 8
UPG = GROUP // UNIT           # units per group = 8
VP = VOCAB + 1                # 96 vocab rows + 1 ones row (bias)
SLOTS = W + 5                 # one-hot w-slots: 2 left pad, 16 chars, 3 right pad
HT = 32                       # tokens per half-unit
NPAIR = 3                     # fp8 DoubleRow tap pairs: (0,1), (2,3), (4,zero)
FPAD = 112                    # fp8 table stride: 16B-aligned (dual-fp8 ldweights rule)
LO_SCALE = 4096.0             # fp8 residual pre-scale; one-hot carries 2^-12

f16 = mybir.dt.float16
f32 = mybir.dt.float32
f32r = mybir.dt.float32r
f8e4 = mybir.dt.float8e4
f8e5 = mybir.dt.float8e5
i32 = mybir.dt.int32


def build_nc():
    nc = bacc.Bacc("TRN2", target_bir_lowering=False)

    # ids pre-transposed on host to [unit, half, (w, t)] so the DMA is flat
    ids_d = nc.dram_tensor("ids", [NUNIT, 2, W * HT], i32, kind="ExternalInput")
    # hi table: G_hi[v, tap k, f] in f32r (12 explicit mantissa bits)
    gtab_d = nc.dram_tensor("gtab", [VP, K, F], f32r, kind="ExternalInput")
    # lo table: fp8(4096 * (G - G_hi)) as DoubleRow pairs [v, pair, ktile, fpad]
    gtab8_d = nc.dram_tensor("gtab8", [VP, NPAIR * 2 * FPAD], f8e4, kind="ExternalInput")
    ones_d = nc.dram_tensor("ones", [33, VOCAB], f32r, kind="ExternalInput")
    out_d = nc.dram_tensor("out", [NGROUP, F, GROUP], f32, kind="ExternalOutput")

    with TileContext(nc) as tc:
        with (
            tc.tile_pool(name="consts", bufs=1) as consts,
            tc.tile_pool(name="outp", bufs=2) as outp,
            tc.tile_pool(name="idsp", bufs=3) as idsp,
            tc.tile_pool(name="psA", bufs=2, space="PSUM") as psA,
            tc.tile_pool(name="psB", bufs=2, space="PSUM") as psB,
        ):
            ids_tiles = {}

            def load_ids(g):
                idst = idsp.tile([33, GROUP * W // 2], f32r, tag="ids")
                v = ids_d[g * UPG : (g + 1) * UPG, :, :]
                nc.gpsimd.dma_start(out=idst[0:1, :], in_=v[:, 0, :])
                nc.gpsimd.dma_start(out=idst[32:33, :], in_=v[:, 1, :])
                ids_tiles[g] = idst

            # gate-first DMAs: the first bcast needs ids + ones
            load_ids(0)
            load_ids(1)
            ones_t = consts.tile([33, VOCAB], f32r)
            nc.gpsimd.dma_start(out=ones_t, in_=ones_d[:, :])

            # per-partition compare values 0..95 generated on-device
            iota_t = consts.tile([VOCAB, 1], f32)
            nc.gpsimd.iota(
                iota_t[:, :], pattern=[[0, 1]], base=0, channel_multiplier=1,
                allow_small_or_imprecise_dtypes=True,
            )
            # touch the DVE with the is_equal opcode early: absorbs the
            # engine's first-dispatch latency during the init phase.
            dve_warm = consts.tile([VOCAB, 1], f32, tag="dve_warm")
            nc.vector.tensor_scalar(
                out=dve_warm[:, :],
                in0=iota_t[:, :],
                scalar1=iota_t[:, 0:1],
                scalar2=None,
                op0=mybir.AluOpType.is_equal,
            )

            # Two persistent one-hot tile sets (ping-pong across units):
            #  o_t: f32r [VP, half, SLOTS, HT] — char position w at slot w+2,
            #       pad slots {0,1,18,19,20} zero, row 96 = 1.0 bias carrier.
            #  o8: e5m2 [VP, half, 2, SLOTS, HT] — two tap-shifted planes of
            #       the one-hot scaled by 2^-12 for the fp8 DoubleRow lo pass
            #       (plane j at slot s == o_t slot s+j).
            o_tiles = []
            o8_tiles = []
            for j in range(2):
                ot = consts.tile([VP, 2, SLOTS, HT], f32r, tag=f"onehot{j}")
                otf = ot.bitcast(f32)
                nc.gpsimd.memset(otf[:, :, 0:2, :].rearrange("v h s t -> v h (s t)"), 0.0)
                nc.gpsimd.memset(
                    otf[:, :, W + 2 : SLOTS, :].rearrange("v h s t -> v h (s t)"), 0.0
                )
                nc.gpsimd.memset(
                    otf[VOCAB : VOCAB + 1, :, 2 : 2 + W, :].rearrange("v h s t -> v h (s t)"),
                    1.0,
                )
                o_tiles.append(ot)
                o8t = consts.tile(
                    [VP, 2, 2, SLOTS, HT], f8e5, tag=f"onehot8{j}", name=f"o8_{j}"
                )
                o8_tiles.append(o8t)

            def bcast(u):
                # broadcast ids across 96 partitions (K=1 matmul) + one-hot
                g, uu = divmod(u, UPG)
                idst = ids_tiles[g]
                bc = psA.tile([VOCAB, 2, W, HT], f32, tag="bcast")
                for h in range(2):
                    p0 = 32 * h
                    nc.tensor.matmul(
                        bc[:, h, :, :],
                        ones_t[p0 : p0 + 1, :],
                        idst[p0 : p0 + 1, uu * 512 : (uu + 1) * 512],
                        start=True,
                        stop=True,
                    )
                # one-hot: O[v, h, w+2, t] = (ids[h, t, w] == v); all access
                # sequential (bc and o_t share (h, w, t) order)
                o_t = o_tiles[u % 2]
                nc.vector.tensor_scalar(
                    out=o_t[0:VOCAB, :, 2 : 2 + W, :],
                    in0=bc[:, :, :, :],
                    scalar1=iota_t[:, 0:1],
                    scalar2=None,
                    op0=mybir.AluOpType.is_equal,
                )
                # fp8 one-hot planes on the Scalar engine: o8[:,h,j,s,:] =
                # o_t[:,h,s+j,:] * 2^-12 (cast to e5m2; 2^-12 is exact there)
                o8 = o8_tiles[u % 2]
                otf = o_t.bitcast(f32)
                nc.scalar.mul(
                    o8[:, :, 0, :, :].rearrange("v h s t -> v h (s t)"),
                    otf[:, :, :, :].rearrange("v h s t -> v h (s t)"),
                    1.0 / LO_SCALE,
                )
                nc.scalar.mul(
                    o8[:, :, 1, 0 : SLOTS - 1, :].rearrange("v h s t -> v h (s t)"),
                    otf[:, :, 1:SLOTS, :].rearrange("v h s t -> v h (s t)"),
                    1.0 / LO_SCALE,
                )

            gtab = consts.tile([VP, K, F], f32r)
            nc.gpsimd.dma_start(
                out=gtab.rearrange("v s f -> v (s f)"),
                in_=gtab_d.rearrange("v s f -> v (s f)"),
            )
            gtab8 = consts.tile([VP, NPAIR, 2, FPAD], f8e4)
            nc.gpsimd.dma_start(
                out=gtab8.rearrange("v p j f -> v (p j f)"),
                in_=gtab8_d[:, :],
            )

            # PE warmup: tiny matmuls keep the HAM activity window busy while
            # the init DMAs land, so real matmuls start at full clock.
            warm = psA.tile([1, 1], f32, tag="bcast")
            for _ in range(48):
                nc.tensor.matmul(
                    warm[0:1, 0:1],
                    iota_t[0:1, 0:1],
                    iota_t[0:1, 0:1],
                    start=True,
                    stop=True,
                )

            bcast(0)
            out_sb = None
            for u in range(NUNIT):
                g, uu = divmod(u, UPG)
                if uu == 0:
                    out_sb = outp.tile([F, GROUP], f32, tag="osb")
                    if g + 2 < NGROUP:
                        load_ids(g + 2)
                # emit next unit's bcast+one-hot BEFORE this unit's taps so the
                # in-order PE queue never stalls waiting on the DVE/ACT.
                if u + 1 < NUNIT:
                    bcast(u + 1)

                o_t = o_tiles[u % 2]
                o8 = o8_tiles[u % 2]
                # hi: 5 taps f32r; lo: 3 fp8 DoubleRow pairs — all PSUM-accum'd
                ys = [psB.tile([F, W, HT], f32, tag=f"y{h}", name=f"y{h}") for h in range(2)]
                for k in range(K):
                    for h in range(2):
                        nc.tensor.matmul(
                            ys[h][:, :, :],
                            gtab[:, k, :],
                            o_t[:, h, k : k + W, :],
                            start=(k == 0),
                            stop=False,
                            skip_group_check=True,
                        )
                for p in range(NPAIR):
                    for h in range(2):
                        nc.tensor.matmul(
                            ys[h][:, :, :],
                            gtab8[:, p, :, 0:F],
                            o8[:, h, :, 2 * p : 2 * p + W, :],
                            start=False,
                            stop=(p == NPAIR - 1),
                            perf_mode=mybir.MatmulPerfMode.DoubleRow,
                            skip_group_check=True,
                        )

                # max over the 16 char positions (w is the outer free dim)
                for h in range(2):
                    nc.vector.reduce_max(
                        out=out_sb[:, uu * UNIT + h * HT : uu * UNIT + (h + 1) * HT],
                        in_=ys[h].rearrange("f w t -> f t w"),
                        axis=mybir.AxisListType.X,
                    )

                if uu == UPG - 1:
                    # stream this group's result out to DRAM (contiguous block)
                    nc.sync.dma_start(out=out_d[g, :, :], in_=out_sb[:, :])

    nc.compile()
    return nc


def _round_f32r(x):
    """FP32R keeps 12 explicit mantissa bits (low 12 bits of fp32 zeroed)."""
    b = np.asarray(x, np.float32).view(np.uint32)
    b = (b + 0x800) & np.uint32(0xFFFFF000)
    return b.view(np.float32)


def make_consts(embed_table, conv_w, conv_b):
    # G[k][v, f] = sum_d E[v, d] * Wc[f, d, k] in float64; hi f32r + lo fp8
    G = np.einsum(
        "vd,fdk->kvf", embed_table.astype(np.float64), conv_w.astype(np.float64)
    )
    Gf = np.zeros((K, VP, F), np.float64)
    Gf[:, 0:VOCAB, :] = G
    Gf[2, VOCAB, :] = conv_b.astype(np.float64)  # bias rides center tap
    hi = _round_f32r(Gf.astype(np.float32))
    e4 = mybir.dt.np(f8e4)
    lo8 = ((Gf - hi.astype(np.float64)) * LO_SCALE).astype(np.float32).astype(e4)
    gtab = np.ascontiguousarray(np.transpose(hi, (1, 0, 2)))  # [VP, K, F]
    gtab8 = np.zeros((VP, NPAIR, 2, FPAD), e4)
    for p in range(NPAIR):
        gtab8[:, p, 0, 0:F] = lo8[2 * p]
        if 2 * p + 1 < K:
            gtab8[:, p, 1, 0:F] = lo8[2 * p + 1]
    ones = np.zeros((33, VOCAB), np.float32)
    ones[0, :] = 1.0
    ones[32, :] = 1.0
    return gtab, gtab8.reshape(VP, NPAIR * 2 * FPAD), ones


_NC_CACHE = {}

# Test-harness knobs (ignored by normal kernel() use)
TRACE = False
LAST_RESULT = None


def kernel(char_ids, embed_table, conv_w, conv_b):
    global LAST_RESULT
    char_ids = np.asarray(char_ids)
    gtab, gtab8, ones = make_consts(
        np.asarray(embed_table), np.asarray(conv_w), np.asarray(conv_b)
    )

    if "nc" not in _NC_CACHE:
        _NC_CACHE["nc"] = build_nc()
    nc = _NC_CACHE["nc"]

    in_maps = []
    for c in range(N_CORES):
        shard = char_ids[c * NSH : (c + 1) * NSH]
        # [unit, half, t, w] -> [unit, half, w, t] so device reads are flat
        shard = np.ascontiguousarray(
            shard.reshape(NUNIT, 2, HT, W).transpose(0, 1, 3, 2).reshape(NUNIT, 2, W * HT)
        )
        in_maps.append({"ids": shard, "gtab": gtab, "gtab8": gtab8, "ones": ones})

    kwargs = {}
    if TRACE:
        kwargs = dict(trace=True, trace_cores=list(range(N_CORES)))
    res = run_bass_kernel_spmd(nc, in_maps, core_ids=list(range(N_CORES)), **kwargs)
    LAST_RESULT = res

    out = np.empty((N, F), np.float32)
    for c in range(N_CORES):
        o = res.results[c]["out"]  # [NGROUP, F, GROUP]
        out[c * NSH : (c + 1) * NSH] = o.transpose(0, 2, 1).reshape(NSH, F)
    return out
